# revision 1
# baseline (speedup 1.0000x reference)
"""Trainium2 Bass kernel for nn_ClinicalEmbedding (EmbeddingBag-style ragged gather).

Semantics (matches reference.py):
  flat = codes.reshape(B, L); g = renorm(W[flat])  (max_norm=1.0)
  out[b, v] = 0                       for v <  V - nv[b]
            = g[b, v - (V-nv[b])]     for V-nv[b] <= v < V-1
            = sum_{j=nv-1}^{nv*C-1} g[b, j]   for v = V-1

Sharding: data-parallel over batch across 8 cores, W replicated.
All data-dependent structure (gather indices, weights/masks) is expressed as
per-core *data*, so a single NEFF runs SPMD on all 8 cores. Chunk counts are
balanced across cores by sorting patients by bag length and snake-dealing.
"""

import math
import os

import numpy as np

import concourse.bacc as bacc
import concourse.bass as bass
import concourse.mybir as mybir
import concourse.tile as tile
from concourse.bass_utils import run_bass_kernel_spmd

P = 128          # SBUF partitions
N_CORES = 8

# group size (chunks of 128 gathered rows) per indirect DMA in the bag pass
G = 13

LAST_RESULTS = None   # test harness reads profiling info from here


def _prepare(codes, nv, B, V, C, L, VOCAB, E):
    """Host-side index/mask construction. Returns static structure + per-core data."""
    B_LOC = B // N_CORES
    nbag = nv * (C - 1) + 1                    # bag length per patient
    nch = (nbag + P - 1) // P                  # gather chunks per patient
    order = np.argsort(-nch, kind="stable")    # rank -> patient, desc by work

    assign = np.empty((N_CORES, B_LOC), dtype=np.int64)
    for r, b in enumerate(order):
        assign[r % N_CORES, r // N_CORES] = b

    # static per-slot chunk count = max over cores = first patient of each rank-group
    S = np.array([nch[order[s * N_CORES]] for s in range(B_LOC)], dtype=np.int64)
    offs = np.concatenate([[0], np.cumsum(S)]).astype(np.int64)
    T = int(offs[-1])
    slot_of_chunk = np.repeat(np.arange(B_LOC), S)

    # ---- bag pass data: idxB [P, T] int32, wB [P, T] f32 ----
    idxB = np.zeros((N_CORES, T, P), np.int32)
    wB = np.zeros((N_CORES, T, P), np.float32)
    for k in range(N_CORES):
        for s in range(B_LOC):
            b = assign[k, s]
            n = int(nv[b])
            nb = n * (C - 1) + 1
            vals = codes[b, n - 1 : n - 1 + nb]
            c0, c1 = offs[s], offs[s + 1]
            cap = int(c1 - c0) * P
            bi = np.zeros(cap, np.int32)
            bw = np.zeros(cap, np.float32)
            bi[:nb] = vals
            bw[:nb] = 1.0
            idxB[k, c0:c1, :] = bi.reshape(-1, P)
            wB[k, c0:c1, :] = bw.reshape(-1, P)
    idxB = np.ascontiguousarray(idxB.transpose(0, 2, 1))   # [cores, P, T]
    wB = np.ascontiguousarray(wB.transpose(0, 2, 1))

    # ---- singles pass data: idxS [P, SC] int32, wS [P, SC] f32 ----
    # entry (p, c): p = b0*(V-1) + v for b0 in {0,1}, v in [0, V-1); slot = 2c + b0
    SC = B_LOC // 2
    idxS = np.zeros((N_CORES, P, SC), np.int32)
    wS = np.zeros((N_CORES, P, SC), np.float32)
    v_arr = np.arange(V - 1)
    for k in range(N_CORES):
        for s in range(B_LOC):
            b = assign[k, s]
            n = int(nv[b])
            b0, c = s % 2, s // 2
            valid = v_arr >= (V - n)
            j = np.clip(v_arr - (V - n), 0, L - 1)
            idxS[k, b0 * (V - 1) + v_arr, c] = np.where(valid, codes[b, j], 0)
            wS[k, b0 * (V - 1) + v_arr, c] = valid.astype(np.float32)

    return dict(
        B_LOC=B_LOC, T=T, offs=offs, slot_of_chunk=slot_of_chunk, SC=SC,
        assign=assign, idxB=idxB, wB=wB, idxS=idxS, wS=wS,
    )


def _build(prep, V, C, VOCAB, E):
    """Emit the Bass/Tile program (shared across all 8 cores)."""
    B_LOC, T, offs, soc, SC = (
        prep["B_LOC"], prep["T"], prep["offs"], prep["slot_of_chunk"], prep["SC"]
    )
    f32 = mybir.dt.float32
    i32 = mybir.dt.int32

    nc = bacc.Bacc("TRN2", num_devices=N_CORES, debug=False)
    W_d = nc.dram_tensor("W", [VOCAB, E], f32, kind="ExternalInput")
    idxB_d = nc.dram_tensor("idxB", [P, T], i32, kind="ExternalInput")
    wB_d = nc.dram_tensor("wB", [P, T], f32, kind="ExternalInput")
    idxS_d = nc.dram_tensor("idxS", [P, SC], i32, kind="ExternalInput")
    wS_d = nc.dram_tensor("wS", [P, SC], f32, kind="ExternalInput")
    out_d = nc.dram_tensor("out", [B_LOC * V, E], f32, kind="ExternalOutput")

    n_groups = math.ceil(T / G)

    with tile.TileContext(nc) as tc:
        with (
            tc.tile_pool(name="const", bufs=1) as cpool,
            tc.tile_pool(name="g", bufs=3) as gpool,
            tc.tile_pool(name="sq", bufs=2) as sqpool,
            tc.tile_pool(name="sm", bufs=2) as smpool,
            tc.tile_pool(name="ps", bufs=1, space="PSUM") as pspool,
        ):
            idxB_t = cpool.tile_from(idxB_d[:])
            wB_t = cpool.tile_from(wB_d[:])
            idxS_t = cpool.tile_from(idxS_d[:])
            wS_t = cpool.tile_from(wS_d[:])

            psum = pspool.tile([1, B_LOC * E], f32)

            # zero bias tile written by DVE so ACT sqrt waits only on DVE
            zbias = smpool.tile([P, 1], f32, tag="zbias", bufs=1)
            nc.vector.memset(zbias[:], 0.0)

            # ---------------- singles pass ----------------
            gS = gpool.tile([P, SC * E], f32, tag="gS", bufs=1)
            for c in range(SC):
                nc.gpsimd.indirect_dma_start(
                    out=gS[:, c * E : (c + 1) * E], out_offset=None, in_=W_d[:],
                    in_offset=bass.IndirectOffsetOnAxis(ap=idxS_t[:, c : c + 1], axis=0),
                )
            sqS = sqpool.tile([P, SC * E], f32, tag="sqS", bufs=1)
            nc.vector.tensor_mul(sqS[:], gS[:], gS[:])
            nS = smpool.tile([P, SC], f32, tag="nS", bufs=1)
            nc.vector.tensor_reduce(
                nS[:], sqS[:].rearrange("p (c e) -> p c e", e=E),
                axis=mybir.AxisListType.X, op=mybir.AluOpType.add,
            )
            nc.vector.tensor_scalar_max(nS[:], nS[:], 1.0)
            sqS2 = smpool.tile([P, SC], f32, tag="sqS2", bufs=1)
            nc.scalar.activation(
                sqS2[:], nS[:], mybir.ActivationFunctionType.Sqrt, bias=zbias[:]
            )
            rS = smpool.tile([P, SC], f32, tag="rS", bufs=1)
            nc.vector.reciprocal(rS[:], sqS2[:])
            nc.vector.tensor_mul(rS[:], rS[:], wS_t[:])
            nc.vector.tensor_tensor(
                out=gS[:].rearrange("p (c e) -> p c e", e=E),
                in0=gS[:].rearrange("p (c e) -> p c e", e=E),
                in1=rS[:].to_broadcast([P, SC, E]),
                op=mybir.AluOpType.mult,
            )
            # store rows (slot=2c+b0, v) <- gS[p=b0*(V-1)+v, block c]
            out_bv = out_d[:].rearrange("(c b0 v) e -> b0 v c e", c=SC, b0=2, v=V)
            for b0 in range(2):
                nc.sync.dma_start(
                    out=out_bv[b0, : V - 1],
                    in_=gS[b0 * (V - 1) : (b0 + 1) * (V - 1), :].rearrange(
                        "p (c e) -> p c e", e=E
                    ),
                )

            # ---------------- bag pass ----------------
            for g in range(n_groups):
                c0, c1 = g * G, min((g + 1) * G, T)
                Gg = c1 - c0
                gB = gpool.tile([P, Gg * E], f32, tag="gB")
                for cl in range(Gg):
                    nc.gpsimd.indirect_dma_start(
                        out=gB[:, cl * E : (cl + 1) * E], out_offset=None, in_=W_d[:],
                        in_offset=bass.IndirectOffsetOnAxis(
                            ap=idxB_t[:, c0 + cl : c0 + cl + 1], axis=0
                        ),
                    )
                sqB = sqpool.tile([P, Gg * E], f32, tag="sqB")
                nc.vector.tensor_mul(sqB[:], gB[:], gB[:])
                nB = smpool.tile([P, Gg], f32, tag="nB")
                nc.vector.tensor_reduce(
                    nB[:], sqB[:].rearrange("p (c e) -> p c e", e=E),
                    axis=mybir.AxisListType.X, op=mybir.AluOpType.add,
                )
                nc.vector.tensor_scalar_max(nB[:], nB[:], 1.0)
                sqB2 = smpool.tile([P, Gg], f32, tag="sqB2")
                nc.scalar.activation(
                    sqB2[:], nB[:], mybir.ActivationFunctionType.Sqrt, bias=zbias[:]
                )
                rB = smpool.tile([P, Gg], f32, tag="rB")
                nc.vector.reciprocal(rB[:], sqB2[:])
                nc.vector.tensor_mul(rB[:], rB[:], wB_t[:, c0:c1])
                for c in range(c0, c1):
                    s = int(soc[c])
                    cl = c - c0
                    nc.tensor.matmul(
                        out=psum[0:1, s * E : (s + 1) * E],
                        lhsT=rB[:, cl : cl + 1],
                        rhs=gB[:, cl * E : (cl + 1) * E],
                        start=(c == offs[s]),
                        stop=(c == offs[s + 1] - 1),
                    )

            outS = smpool.tile([1, B_LOC * E], f32, tag="outS", bufs=1)
            nc.vector.tensor_copy(outS[:], psum[:])
            nc.sync.dma_start(
                out=out_d[:].rearrange("(b v) e -> b v e", v=V)[:, V - 1, :],
                in_=outS[:].rearrange("p (b e) -> p b e", e=E),
            )

    nc.compile()   # bacc passes: wait-splitting (<=1 wait/instr on TRN2), nop fusion
    return nc


def kernel(**inputs) -> np.ndarray:
    global LAST_RESULTS
    W = np.ascontiguousarray(np.asarray(inputs["W"], dtype=np.float32))
    codes_in = np.asarray(inputs["codes"])
    nv = np.asarray(inputs["n_visits"]).astype(np.int64)

    B, V, C = codes_in.shape
    VOCAB, E = W.shape
    L = V * C
    codes = np.ascontiguousarray(codes_in.reshape(B, L).astype(np.int32))

    prep = _prepare(codes, nv, B, V, C, L, VOCAB, E)
    nc = _build(prep, V, C, VOCAB, E)

    in_maps = [
        {
            "W": W,
            "idxB": prep["idxB"][k],
            "wB": prep["wB"][k],
            "idxS": prep["idxS"][k],
            "wS": prep["wS"][k],
        }
        for k in range(N_CORES)
    ]
    trace = bool(int(os.environ.get("KERNEL_TRACE", "0")))
    res = run_bass_kernel_spmd(
        nc, in_maps, core_ids=list(range(N_CORES)), trace=trace
    )
    LAST_RESULTS = res

    B_LOC = prep["B_LOC"]
    assign = prep["assign"]
    full = np.zeros((B, V, E), np.float32)
    for k in range(N_CORES):
        o = res.results[k]["out"].reshape(B_LOC, V, E)
        full[assign[k]] = o
    return full

